# revision 6
# baseline (speedup 1.0000x reference)
"""Trainium2 Bass kernel v3 for nn_CRAP_16544214024675 (sparse_attention).

Reference computation (per batch b, channel c):
  q = Wq@feat + bq                        (1x1 conv over channels)
  k = unfold3x3_s2(src)                   (strided window gather, pad 1)
  v = unfold3x3_s2(Wv@src + bv)
  A = softmax_t( sum_px q*k_t / 64 )      (9 window positions)
  out = fold3x3_s1( A_t * v_t ) * feat

Sharding: 8 cores = 4 batches x 2 output-channel halves (slot 0 of packed
inputs = the core's own half; channel sums commute).

v3 vs v2 (79.6us baseline): full reschedule for overlap.
 - DMA: featb as 4 row-band transfers (early q-conv start), then src
   parity planes ct0-first (products are the long pole), ct1 planes after,
   ct1 of plane (0,0) last (its vplane feeds only the late fold passes).
 - products: 7 DVE STT + 2 gpsimd mul (plane-(1,1) t's) with ACT reduces.
   No shifted-q copy (STT runs 1x, no alignment constraint).
 - fold in two half-image PSUM phases (4 banks each): half0 coexists with
   the conv chunk pool (2x2 banks) and accumulates t-by-t as exps arrive;
   half1 reuses the conv banks after convs drain. Zero-prefill per half
   keeps accumulation order free.
 - v-convs K-split: ct0 matmuls run as soon as the ct0 plane lands, ct1
   matmuls chase the late DMA.
 - final out = (fold*rz)*feat per 16-row band: bands 0-1 via ACT scale +
   DVE mul, bands 2-3 via one DVE STT each; band DMA-out chases.
"""
import sys
from contextlib import ExitStack

import numpy as np

for _p in ("/opt/trn_rl_repo", "/root/.axon_site/_ro/trn_rl_repo"):
    if _p not in sys.path:
        sys.path.append(_p)

import ml_dtypes

import concourse.tile as tile
from concourse import bacc, mybir
from concourse import bass_utils
from concourse.bass_interp import get_hw_module

F32 = mybir.dt.float32
BF16 = mybir.dt.bfloat16
AF = mybir.ActivationFunctionType
ALU = mybir.AluOpType

B, C, H, W = 4, 256, 64, 64
N_CORES = 8

# src parity plane (p,q) -> t's whose product reads its ct0 and whose fold
# reads vplane (p,q)
PLANE_TS = {
    (0, 0): [(1, 1)],
    (1, 1): [(2, 2), (0, 0), (0, 2), (2, 0)],
    (1, 0): [(0, 1), (2, 1)],
    (0, 1): [(1, 0), (1, 2)],
}
# DMA arrival order of (plane, ct): all ct0 first (product chain), then
# ct1; plane (0,0) ct1 last (only feeds the final fold passes), split in 2.
DMA_PLANES = [
    ((0, 0), 0), ((1, 1), 0), ((1, 0), 0), ((0, 1), 0),
    ((1, 1), 1), ((1, 0), 1), ((0, 1), 1), ((0, 0), 1),
]
CONV_ORDER = [(1, 1), (1, 0), (0, 1), (0, 0)]
# per-t product engine: 'S' DVE STT fused; 'G' gpsimd mul + ACT reduce
T_TYPE = {
    (1, 1): 'S', (2, 2): 'S', (0, 0): 'G', (0, 2): 'S', (2, 0): 'G',
    (0, 1): 'S', (2, 1): 'S', (1, 0): 'S', (1, 2): 'S',
}
# product emission order (DVE/GPS priority); follows ct0 arrival
T_ORDER = [(1, 1), (2, 2), (0, 0), (2, 0), (0, 2),
           (0, 1), (2, 1), (1, 0), (1, 2)]
T_IDX = {t: i for i, t in enumerate(T_ORDER)}
# fold emission order per half = expected readiness order (WAW serialized)
FOLD_ORDER = [(2, 2), (0, 0), (0, 2), (2, 0), (0, 1),
              (2, 1), (1, 0), (1, 2), (1, 1)]
N_WARM = 30


def _prod_views(i, j):
    qr0, pr0, rows = (1, 0, 63) if i == 0 else (0, 0, 64)
    qc0, pc0, cols = (1, 0, 63) if j == 0 else (0, 0, 64)
    return qr0, pr0, rows, qc0, pc0, cols


def _windows(i, j):
    if i == 0:
        yo0, yo1, dy = 0, 63, 0
    elif i == 1:
        yo0, yo1, dy = 0, 64, 0
    else:
        yo0, yo1, dy = 1, 64, -1
    if j == 0:
        xo0, xo1, dx = 0, 63, 0
    elif j == 1:
        xo0, xo1, dx = 0, 64, 0
    else:
        xo0, xo1, dx = 1, 64, -1
    return yo0, yo1, dy, xo0, xo1, dx


def build_program():
    nc = bacc.Bacc("TRN2", target_bir_lowering=False, debug=False)

    fb_d = nc.dram_tensor("featbands", (4, 2, 128, 16, W), BF16,
                          kind="ExternalInput")
    spl_d = nc.dram_tensor("splanes", (8, 128, H, W), BF16,
                           kind="ExternalInput")
    wq_d = nc.dram_tensor("wq", (2, 128, 128), BF16, kind="ExternalInput")
    wv_d = nc.dram_tensor("wv", (2, 128, 128), BF16, kind="ExternalInput")
    bq_d = nc.dram_tensor("bq", (128, 1), F32, kind="ExternalInput")
    bv_d = nc.dram_tensor("bv", (128, 1), F32, kind="ExternalInput")
    id_d = nc.dram_tensor("identb", (128, 128), BF16, kind="ExternalInput")
    out_d = nc.dram_tensor("out", (128, H, W), BF16, kind="ExternalOutput")

    with tile.TileContext(nc) as tc, ExitStack() as ctx:
        pool = ctx.enter_context(tc.tile_pool(name="main", bufs=1))
        scpool = ctx.enter_context(tc.tile_pool(name="scratch", bufs=3))
        wtpool = ctx.enter_context(tc.tile_pool(name="wts", bufs=9))

        # --- input DMA: emission order = priority order ---
        wq_t = pool.tile([128, 2, 128], BF16, tag="wq")
        wv_t = pool.tile([128, 2, 128], BF16, tag="wv")
        bq_t = pool.tile([128, 1], F32, tag="bq")
        bv_t = pool.tile([128, 1], F32, tag="bv")
        nc.sync.dma_start(wq_t[:], wq_d.ap().rearrange("a p b -> p a b"))
        nc.sync.dma_start(bq_t[:], bq_d.ap())
        fbt = []
        for b in range(4):
            t_ = pool.tile([128, 2, 16, W], BF16, tag=f"fb{b}")
            nc.sync.dma_start(t_[:], fb_d.ap()[b].rearrange("h p r c -> p h r c"))
            fbt.append(t_)
        nc.sync.dma_start(wv_t[:], wv_d.ap().rearrange("a p b -> p a b"))
        nc.sync.dma_start(bv_t[:], bv_d.ap())
        id_t = pool.tile([128, 128], BF16, tag="identb")
        nc.sync.dma_start(id_t[:], id_d.ap())
        splane = {}
        for si, (pq, ct) in enumerate(DMA_PLANES):
            t_ = pool.tile([128, H, W], BF16, tag=f"spl{ct}{pq[0]}{pq[1]}",
                           name=f"spl{ct}{pq[0]}{pq[1]}")
            splane[(ct,) + pq] = t_
            if si == len(DMA_PLANES) - 1:
                nc.sync.dma_start(t_[:, 0:32, :], spl_d.ap()[si, :, 0:32, :])
                nc.sync.dma_start(t_[:, 32:64, :], spl_d.ap()[si, :, 32:64, :])
            else:
                nc.sync.dma_start(t_[:], spl_d.ap()[si])

        lg_t = pool.tile([128, 16], F32, tag="lg")
        exp_t = pool.tile([128, 16], F32, tag="exp")
        q_t = pool.tile([128, H, W], BF16, tag="q")
        zero_t = pool.tile([128, 128], BF16, tag="zero")
        nc.gpsimd.memset(zero_t[:], 0.0)
        warm_t = pool.tile([128, 256], BF16, tag="warm")
        nc.gpsimd.memset(warm_t[:], 0.5)

        # trigger the ACT exp-table load immediately (off the critical path)
        nc.gpsimd.memset(lg_t[:, 15:16], 0.0)
        with tc.high_priority():
            nc.scalar.activation(exp_t[:, 15:16], lg_t[:, 15:16], AF.Exp,
                                 scale=1.0 / 64.0)

        # PSUM: conv chunks 2x[128,16,W] (4 banks) + fold half0 (4 banks);
        # fold half1 opens after the conv pool closes.
        psc_cm = tc.tile_pool(name="psc", bufs=2, space="PSUM")
        psfA = ctx.enter_context(
            tc.tile_pool(name="psfA", bufs=1, space="PSUM"))
        fold_h = [psfA.tile([128, 32, W], F32, tag="foldh0", name="foldh0")]

        def fold_prefill(fps):
            for s in range(4):
                nc.tensor.matmul(
                    fps[:, 8 * s:8 * s + 8, :], zero_t[:],
                    splane[(0, 0, 0)][:, 8 * s:8 * s + 8, :],
                    start=True, stop=False, skip_group_check=True,
                )

        def fold_pass(t_ij, half, stop):
            i, j = t_ij
            yo0, yo1, dy, xo0, xo1, dx = _windows(i, j)
            pq = ((i + 1) % 2, (j + 1) % 2)
            vp = vplane[pq]
            dg = diags[(i, j)]
            fps = fold_h[half]
            r0 = 32 * half
            yb = max(yo0, r0)
            yend = min(yo1, r0 + 32)
            while yb < yend:
                ye = min((yb // 8 + 1) * 8, yend)
                nc.tensor.matmul(
                    fps[:, yb - r0:ye - r0, xo0:xo1],
                    dg[:],
                    vp[:, yb + dy:ye + dy, xo0 + dx:xo1 + dx],
                    start=False, stop=stop,
                    skip_group_check=True,
                )
                yb = ye

        vplane = {}
        diags = {}
        with psc_cm as psc:
            # PE warm-up burst (dependency-free) to ramp the clock
            wps = psc.tile([128, 16, W], F32, tag="cv", name="warmps")
            for w_i in range(N_WARM):
                nc.tensor.matmul(
                    wps[:, 0:2, :], warm_t[:, 0:128], warm_t[:, 128:256],
                    start=True, stop=True, skip_group_check=True,
                )

            # q-conv: 4 row-band chunks chasing the featb DMAs
            for b in range(4):
                ps = psc.tile([128, 16, W], F32, tag="cv", name=f"qps{b}")
                for s in range(2):
                    for k in range(2):
                        nc.tensor.matmul(
                            ps[:, 8 * s:8 * s + 8, :],
                            wq_t[:, k, :],
                            fbt[b][:, k, 8 * s:8 * s + 8, :],
                            start=(k == 0), stop=(k == 1),
                        )
                with tc.high_priority(offset=60):
                    nc.scalar.activation(q_t[:, 16 * b:16 * b + 16, :],
                                         ps[:], AF.Identity, bias=bq_t[:])

            # fold half0 prefill early (PE is otherwise waiting on DMA)
            fold_prefill(fold_h[0])

            # --- products ---
            def product(t_ij):
                i, j = t_ij
                idx = T_IDX[t_ij]
                pq = [k for k, ts in PLANE_TS.items() if t_ij in ts][0]
                pl = splane[(0,) + pq]
                qr0, pr0, rows, qc0, pc0, cols = _prod_views(i, j)
                q_view = q_t[:, qr0:qr0 + rows, qc0:qc0 + cols]
                p_view = pl[:, pr0:pr0 + rows, pc0:pc0 + cols]
                sc = scpool.tile([128, H, W], BF16, tag="prod",
                                 name=f"prod{idx}")
                if T_TYPE[t_ij] == 'G':
                    nc.gpsimd.tensor_mul(sc[:, 0:rows, 0:cols],
                                         q_view, p_view)
                    with tc.high_priority(offset=50):
                        nc.scalar.activation(
                            sc[:, 0:rows, 0:cols], sc[:, 0:rows, 0:cols],
                            AF.Copy, accum_out=lg_t[:, idx:idx + 1])
                else:
                    with tc.high_priority(offset=50):
                        nc.vector.scalar_tensor_tensor(
                            out=sc[:, 0:rows, 0:cols], in0=q_view,
                            scalar=1.0, in1=p_view,
                            op0=ALU.mult, op1=ALU.mult,
                            accum_out=lg_t[:, idx:idx + 1])
                with tc.high_priority():
                    nc.scalar.activation(
                        exp_t[:, idx:idx + 1], lg_t[:, idx:idx + 1],
                        AF.Exp, scale=1.0 / 64.0)
                dg = wtpool.tile([128, 128], BF16, tag="wt",
                                 name=f"dg{idx}")
                diags[t_ij] = dg
                with tc.high_priority():
                    nc.vector.tensor_scalar(
                        out=dg[:], in0=id_t[:],
                        scalar1=exp_t[:, idx:idx + 1], scalar2=None,
                        op0=ALU.mult)

            for t_ij in T_ORDER:
                product(t_ij)

            # --- v-convs (K-split: ct0 part early, ct1 part chases DMA) ---
            def conv_plane(pq):
                vp = pool.tile([128, H, W], BF16, tag=f"vpl{pq[0]}{pq[1]}",
                               name=f"vpl{pq[0]}{pq[1]}")
                vplane[pq] = vp
                for half in range(2):
                    ps = psc.tile([128, 16, W], F32, tag="cv",
                                  name=f"cv{pq[0]}{pq[1]}h{half}a")
                    ps2 = psc.tile([128, 16, W], F32, tag="cv",
                                   name=f"cv{pq[0]}{pq[1]}h{half}b")
                    for ci, ps_ in enumerate((ps, ps2)):
                        r0 = 32 * half + 16 * ci
                        for s in range(2):
                            for k in range(2):
                                nc.tensor.matmul(
                                    ps_[:, 8 * s:8 * s + 8, :],
                                    wv_t[:, k, :],
                                    splane[(k,) + pq][
                                        :, r0 + 8 * s:r0 + 8 * s + 8, :],
                                    start=(k == 0), stop=(k == 1),
                                )
                    with tc.high_priority(offset=30):
                        nc.scalar.activation(
                            vp[:, 32 * half:32 * half + 16, :], ps[:],
                            AF.Identity, bias=bv_t[:])
                        nc.scalar.activation(
                            vp[:, 32 * half + 16:32 * half + 32, :], ps2[:],
                            AF.Identity, bias=bv_t[:])

            for pq in CONV_ORDER:
                conv_plane(pq)

            # fold half0 passes (t-major, WAW-ordered by emission)
            for t_ij in FOLD_ORDER:
                fold_pass(t_ij, 0, stop=(t_ij == FOLD_ORDER[-1]))

        # conv pool closed -> fold half1 banks free
        psfB = ctx.enter_context(
            tc.tile_pool(name="psfB", bufs=1, space="PSUM"))
        fold_h.append(psfB.tile([128, 32, W], F32, tag="foldh1", name="foldh1"))
        for s in range(4):
            nc.tensor.matmul(
                fold_h[1][:, 8 * s:8 * s + 8, :], zero_t[:],
                splane[(0, 0, 0)][:, 32 + 8 * s:32 + 8 * s + 8, :],
                start=True, stop=False, skip_group_check=True,
            )
        for t_ij in FOLD_ORDER:
            fold_pass(t_ij, 1, stop=(t_ij == FOLD_ORDER[-1]))

        # --- normalization + final: out = (fold * rz) * feat, per band ---
        z_t = pool.tile([128, 8], F32, tag="z")
        rz_t = pool.tile([128, 1], F32, tag="rz")
        with tc.high_priority():
            nc.vector.tensor_reduce(z_t[:, 0:1], exp_t[:, 0:9],
                                    axis=mybir.AxisListType.X, op=ALU.add)
            nc.vector.reciprocal(rz_t[:], z_t[:, 0:1])
        out_t = pool.tile([128, H, W], BF16, tag="out")
        scf = [pool.tile([128, 16, W], BF16, tag=f"scf{i}", name=f"scf{i}") for i in range(2)]
        for band in range(4):
            half, sub = divmod(band, 2)
            fview = fold_h[half][:, 16 * sub:16 * sub + 16, :]
            feat_b = fbt[band][:, 0, :, :]
            oview = out_t[:, 16 * band:16 * band + 16, :]
            if half == 0:
                with tc.high_priority(offset=20):
                    nc.scalar.activation(scf[sub][:], fview,
                                         AF.Identity, scale=rz_t[:])
                    nc.vector.tensor_mul(oview, scf[sub][:], feat_b)
            else:
                with tc.high_priority(offset=20):
                    nc.vector.scalar_tensor_tensor(
                        out=oview, in0=fview, scalar=rz_t[:, 0:1],
                        in1=feat_b, op0=ALU.mult, op1=ALU.mult)
            nc.sync.dma_start(out_d.ap()[:, 16 * band:16 * band + 16, :],
                              oview)

    nc.compile()
    nc.m = get_hw_module(nc.m)
    return nc


_PROGRAM = None


def _get_program():
    global _PROGRAM
    if _PROGRAM is None:
        _PROGRAM = build_program()
    return _PROGRAM


def _prep_inputs(feat, src, Wq, bq, Wv, bv):
    bf = ml_dtypes.bfloat16
    # src parity planes: (B, ct, p, q, 128, H, W); ct0 = own half
    spl = np.ascontiguousarray(
        src.reshape(B, 2, 128, H, 2, W, 2).transpose(0, 1, 4, 6, 2, 3, 5)
    ).astype(bf)
    featb = feat.reshape(B, 2, 128, H, W).astype(bf)
    wq3 = np.ascontiguousarray(Wq.T).reshape(2, 128, C)
    wv3 = np.ascontiguousarray(Wv.T).reshape(2, 128, C)
    identb = np.eye(128, dtype=np.float32).astype(bf)
    in_maps = []
    for core in range(N_CORES):
        b, h = divmod(core, 2)
        oc = slice(h * 128, h * 128 + 128)
        order = [h, 1 - h]  # slot 0 = own input-channel half
        # featbands: (4, 2, 128, 16, W) - row-band major, slot0 = own half
        fb = np.ascontiguousarray(
            featb[b][order].reshape(2, 128, 4, 16, W).transpose(2, 0, 1, 3, 4))
        spl_core = np.stack([
            spl[b][order[ct]][p][q] for (p, q), ct in DMA_PLANES
        ])
        in_maps.append(
            dict(
                featbands=fb,
                splanes=np.ascontiguousarray(spl_core),
                wq=np.ascontiguousarray(wq3[order][:, :, oc]).astype(bf),
                wv=np.ascontiguousarray(wv3[order][:, :, oc]).astype(bf),
                bq=bq[oc].reshape(128, 1).astype(np.float32),
                bv=bv[oc].reshape(128, 1).astype(np.float32),
                identb=identb,
            )
        )
    return in_maps


def kernel(feat, src, Wq, bq, Wv, bv, _trace=False):
    feat = np.asarray(feat, np.float32)
    src = np.asarray(src, np.float32)
    Wq = np.asarray(Wq, np.float32)
    bq = np.asarray(bq, np.float32)
    Wv = np.asarray(Wv, np.float32)
    bv = np.asarray(bv, np.float32)

    in_maps = _prep_inputs(feat, src, Wq, bq, Wv, bv)
    nc = _get_program()
    res = bass_utils.run_bass_kernel_spmd(
        nc, in_maps, core_ids=list(range(N_CORES)), trace=_trace
    )
    out = np.empty((B, C, H, W), np.float32)
    for core in range(N_CORES):
        b, h = divmod(core, 2)
        out[b, h * 128:h * 128 + 128] = np.asarray(
            res.results[core]["out"], dtype=np.float32)
    if _trace:
        kernel.last_results = res
    return out


kernel.last_results = None


# revision 7
# speedup vs baseline: 1.0601x; 1.0601x over previous
"""Trainium2 Bass kernel v3 for nn_CRAP_16544214024675 (sparse_attention).

Reference computation (per batch b, channel c):
  q = Wq@feat + bq                        (1x1 conv over channels)
  k = unfold3x3_s2(src)                   (strided window gather, pad 1)
  v = unfold3x3_s2(Wv@src + bv)
  A = softmax_t( sum_px q*k_t / 64 )      (9 window positions)
  out = fold3x3_s1( A_t * v_t ) * feat

Sharding: 8 cores = 4 batches x 2 output-channel halves (slot 0 of packed
inputs = the core's own half; channel sums commute).

v3 vs v2 (79.6us baseline): full reschedule for overlap.
 - DMA: featb as 4 row-band transfers (early q-conv start), then src
   parity planes ct0-first (products are the long pole), ct1 planes after,
   ct1 of plane (0,0) last (its vplane feeds only the late fold passes).
 - products: 7 DVE STT + 2 gpsimd mul (plane-(1,1) t's) with ACT reduces.
   No shifted-q copy (STT runs 1x, no alignment constraint).
 - fold in two half-image PSUM phases (4 banks each): half0 coexists with
   the conv chunk pool (2x2 banks) and accumulates t-by-t as exps arrive;
   half1 reuses the conv banks after convs drain. Zero-prefill per half
   keeps accumulation order free.
 - v-convs K-split: ct0 matmuls run as soon as the ct0 plane lands, ct1
   matmuls chase the late DMA.
 - final out = (fold*rz)*feat per 16-row band: bands 0-1 via ACT scale +
   DVE mul, bands 2-3 via one DVE STT each; band DMA-out chases.
"""
import sys
from contextlib import ExitStack

import numpy as np

for _p in ("/opt/trn_rl_repo", "/root/.axon_site/_ro/trn_rl_repo"):
    if _p not in sys.path:
        sys.path.append(_p)

import ml_dtypes

import concourse.tile as tile
from concourse import bacc, mybir
from concourse import bass_utils
from concourse.bass_interp import get_hw_module

F32 = mybir.dt.float32
BF16 = mybir.dt.bfloat16
AF = mybir.ActivationFunctionType
ALU = mybir.AluOpType

B, C, H, W = 4, 256, 64, 64
N_CORES = 8

# src parity plane (p,q) -> t's whose product reads its ct0 and whose fold
# reads vplane (p,q)
PLANE_TS = {
    (0, 0): [(1, 1)],
    (1, 1): [(2, 2), (0, 0), (0, 2), (2, 0)],
    (1, 0): [(0, 1), (2, 1)],
    (0, 1): [(1, 0), (1, 2)],
}
# DMA arrival order of (plane, ct): all ct0 first (product chain), then
# ct1; plane (0,0) ct1 last (only feeds the final fold passes), split in 2.
DMA_PLANES = [
    ((0, 0), 0), ((1, 1), 0), ((1, 0), 0), ((0, 1), 0),
    ((1, 1), 1), ((1, 0), 1), ((0, 1), 1), ((0, 0), 1),
]
CONV_ORDER = [(1, 1), (1, 0), (0, 1), (0, 0)]
# per-t product engine: 'S' DVE STT fused; 'A' DVE 2x mul + ACT reduce
# (A-type t's have j in {1,2} so both product views stay alignable)
T_TYPE = {
    (1, 1): 'S', (2, 2): 'A', (0, 0): 'S', (0, 2): 'S', (2, 0): 'S',
    (0, 1): 'A', (2, 1): 'A', (1, 0): 'S', (1, 2): 'S',
}
# product emission order (DVE/GPS priority); follows ct0 arrival
T_ORDER = [(1, 1), (2, 2), (0, 0), (2, 0), (0, 2),
           (0, 1), (2, 1), (1, 0), (1, 2)]
T_IDX = {t: i for i, t in enumerate(T_ORDER)}
# fold emission order per half = expected readiness order (WAW serialized)
FOLD_ORDER = [(2, 2), (0, 0), (0, 2), (2, 0), (0, 1),
              (2, 1), (1, 0), (1, 2), (1, 1)]
N_WARM = 30


def _prod_views(i, j):
    qr0, pr0, rows = (1, 0, 63) if i == 0 else (0, 0, 64)
    qc0, pc0, cols = (1, 0, 63) if j == 0 else (0, 0, 64)
    return qr0, pr0, rows, qc0, pc0, cols


def _windows(i, j):
    if i == 0:
        yo0, yo1, dy = 0, 63, 0
    elif i == 1:
        yo0, yo1, dy = 0, 64, 0
    else:
        yo0, yo1, dy = 1, 64, -1
    if j == 0:
        xo0, xo1, dx = 0, 63, 0
    elif j == 1:
        xo0, xo1, dx = 0, 64, 0
    else:
        xo0, xo1, dx = 1, 64, -1
    return yo0, yo1, dy, xo0, xo1, dx


def build_program():
    nc = bacc.Bacc("TRN2", target_bir_lowering=False, debug=False)

    fb_d = nc.dram_tensor("featbands", (4, 2, 128, 16, W), BF16,
                          kind="ExternalInput")
    spl_d = nc.dram_tensor("splanes", (8, 128, H, W), BF16,
                           kind="ExternalInput")
    wq_d = nc.dram_tensor("wq", (2, 128, 128), BF16, kind="ExternalInput")
    wv_d = nc.dram_tensor("wv", (2, 128, 128), BF16, kind="ExternalInput")
    bq_d = nc.dram_tensor("bq", (128, 1), F32, kind="ExternalInput")
    bv_d = nc.dram_tensor("bv", (128, 1), F32, kind="ExternalInput")
    id_d = nc.dram_tensor("identb", (128, 128), BF16, kind="ExternalInput")
    out_d = nc.dram_tensor("out", (128, H, W), BF16, kind="ExternalOutput")

    with tile.TileContext(nc) as tc, ExitStack() as ctx:
        pool = ctx.enter_context(tc.tile_pool(name="main", bufs=1))
        scpool = ctx.enter_context(tc.tile_pool(name="scratch", bufs=3))
        wtpool = ctx.enter_context(tc.tile_pool(name="wts", bufs=9))

        # --- input DMA: emission order = priority order ---
        wq_t = pool.tile([128, 2, 128], BF16, tag="wq")
        wv_t = pool.tile([128, 2, 128], BF16, tag="wv")
        bq_t = pool.tile([128, 1], F32, tag="bq")
        bv_t = pool.tile([128, 1], F32, tag="bv")
        nc.sync.dma_start(wq_t[:], wq_d.ap().rearrange("a p b -> p a b"))
        nc.sync.dma_start(bq_t[:], bq_d.ap())
        fbt = []
        for b in range(4):
            t_ = pool.tile([128, 2, 16, W], BF16, tag=f"fb{b}")
            nc.sync.dma_start(t_[:], fb_d.ap()[b].rearrange("h p r c -> p h r c"))
            fbt.append(t_)
        nc.sync.dma_start(wv_t[:], wv_d.ap().rearrange("a p b -> p a b"))
        nc.sync.dma_start(bv_t[:], bv_d.ap())
        id_t = pool.tile([128, 128], BF16, tag="identb")
        nc.sync.dma_start(id_t[:], id_d.ap())
        splane = {}
        for si, (pq, ct) in enumerate(DMA_PLANES):
            t_ = pool.tile([128, H, W], BF16, tag=f"spl{ct}{pq[0]}{pq[1]}",
                           name=f"spl{ct}{pq[0]}{pq[1]}")
            splane[(ct,) + pq] = t_
            if si == len(DMA_PLANES) - 1:
                nc.sync.dma_start(t_[:, 0:32, :], spl_d.ap()[si, :, 0:32, :])
                nc.sync.dma_start(t_[:, 32:64, :], spl_d.ap()[si, :, 32:64, :])
            else:
                nc.sync.dma_start(t_[:], spl_d.ap()[si])

        lg_t = pool.tile([128, 16], F32, tag="lg")
        exp_t = pool.tile([128, 16], F32, tag="exp")
        q_t = pool.tile([128, H, W], BF16, tag="q")
        zero_t = pool.tile([128, 128], BF16, tag="zero")
        nc.gpsimd.memset(zero_t[:], 0.0)
        warm_t = pool.tile([128, 256], BF16, tag="warm")
        nc.gpsimd.memset(warm_t[:], 0.5)

        # trigger the ACT exp-table load immediately (off the critical path)
        nc.gpsimd.memset(lg_t[:, 15:16], 0.0)
        with tc.high_priority():
            nc.scalar.activation(exp_t[:, 15:16], lg_t[:, 15:16], AF.Exp,
                                 scale=1.0 / 64.0)

        # PSUM: conv chunks 2x[128,16,W] (4 banks) + fold half0 (4 banks);
        # fold half1 opens after the conv pool closes.
        psc_cm = tc.tile_pool(name="psc", bufs=2, space="PSUM")
        psfA = ctx.enter_context(
            tc.tile_pool(name="psfA", bufs=1, space="PSUM"))
        fold_h = [psfA.tile([128, 32, W], F32, tag="foldh0", name="foldh0")]

        def fold_prefill(fps):
            for s in range(4):
                nc.tensor.matmul(
                    fps[:, 8 * s:8 * s + 8, :], zero_t[:],
                    splane[(0, 0, 0)][:, 8 * s:8 * s + 8, :],
                    start=True, stop=False, skip_group_check=True,
                )

        def fold_pass(t_ij, half, stop):
            i, j = t_ij
            yo0, yo1, dy, xo0, xo1, dx = _windows(i, j)
            pq = ((i + 1) % 2, (j + 1) % 2)
            vp = vplane[pq]
            dg = diags[(i, j)]
            fps = fold_h[half]
            r0 = 32 * half
            yb = max(yo0, r0)
            yend = min(yo1, r0 + 32)
            while yb < yend:
                ye = min((yb // 8 + 1) * 8, yend)
                nc.tensor.matmul(
                    fps[:, yb - r0:ye - r0, xo0:xo1],
                    dg[:],
                    vp[:, yb + dy:ye + dy, xo0 + dx:xo1 + dx],
                    start=False, stop=stop,
                    skip_group_check=True,
                )
                yb = ye

        vplane = {}
        diags = {}
        with psc_cm as psc:
            # PE warm-up burst (dependency-free) to ramp the clock
            wps = psc.tile([128, 16, W], F32, tag="cv", name="warmps")
            for w_i in range(N_WARM):
                nc.tensor.matmul(
                    wps[:, 0:2, :], warm_t[:, 0:128], warm_t[:, 128:256],
                    start=True, stop=True, skip_group_check=True,
                )

            # q-conv: 4 row-band chunks chasing the featb DMAs
            for b in range(4):
                ps = psc.tile([128, 16, W], F32, tag="cv", name=f"qps{b}")
                for s in range(2):
                    for k in range(2):
                        nc.tensor.matmul(
                            ps[:, 8 * s:8 * s + 8, :],
                            wq_t[:, k, :],
                            fbt[b][:, k, 8 * s:8 * s + 8, :],
                            start=(k == 0), stop=(k == 1),
                        )
                with tc.high_priority(offset=60):
                    nc.scalar.activation(q_t[:, 16 * b:16 * b + 16, :],
                                         ps[:], AF.Identity, bias=bq_t[:])

            # fold half0 prefill early (PE is otherwise waiting on DMA)
            fold_prefill(fold_h[0])

            # --- products ---
            def product(t_ij):
                i, j = t_ij
                idx = T_IDX[t_ij]
                pq = [k for k, ts in PLANE_TS.items() if t_ij in ts][0]
                pl = splane[(0,) + pq]
                qr0, pr0, rows, qc0, pc0, cols = _prod_views(i, j)
                q_view = q_t[:, qr0:qr0 + rows, qc0:qc0 + cols]
                p_view = pl[:, pr0:pr0 + rows, pc0:pc0 + cols]
                sc = scpool.tile([128, H, W], BF16, tag="prod",
                                 name=f"prod{idx}")
                if T_TYPE[t_ij] == 'A':
                    with tc.high_priority(offset=50):
                        nc.vector.tensor_mul(sc[:, 0:rows, 0:cols],
                                             q_view, p_view)
                    with tc.high_priority(offset=50):
                        nc.scalar.activation(
                            sc[:, 0:rows, 0:cols], sc[:, 0:rows, 0:cols],
                            AF.Copy, accum_out=lg_t[:, idx:idx + 1])
                else:
                    with tc.high_priority(offset=50):
                        nc.vector.scalar_tensor_tensor(
                            out=sc[:, 0:rows, 0:cols], in0=q_view,
                            scalar=1.0, in1=p_view,
                            op0=ALU.mult, op1=ALU.mult,
                            accum_out=lg_t[:, idx:idx + 1])
                with tc.high_priority():
                    nc.scalar.activation(
                        exp_t[:, idx:idx + 1], lg_t[:, idx:idx + 1],
                        AF.Exp, scale=1.0 / 64.0)
                dg = wtpool.tile([128, 128], BF16, tag="wt",
                                 name=f"dg{idx}")
                diags[t_ij] = dg
                with tc.high_priority():
                    nc.vector.tensor_scalar(
                        out=dg[:], in0=id_t[:],
                        scalar1=exp_t[:, idx:idx + 1], scalar2=None,
                        op0=ALU.mult)

            for t_ij in T_ORDER:
                product(t_ij)

            # --- v-convs (K-split: ct0 part early, ct1 part chases DMA) ---
            def conv_plane(pq):
                vp = pool.tile([128, H, W], BF16, tag=f"vpl{pq[0]}{pq[1]}",
                               name=f"vpl{pq[0]}{pq[1]}")
                vplane[pq] = vp
                for half in range(2):
                    ps = psc.tile([128, 16, W], F32, tag="cv",
                                  name=f"cv{pq[0]}{pq[1]}h{half}a")
                    ps2 = psc.tile([128, 16, W], F32, tag="cv",
                                   name=f"cv{pq[0]}{pq[1]}h{half}b")
                    for ci, ps_ in enumerate((ps, ps2)):
                        r0 = 32 * half + 16 * ci
                        for s in range(2):
                            for k in range(2):
                                nc.tensor.matmul(
                                    ps_[:, 8 * s:8 * s + 8, :],
                                    wv_t[:, k, :],
                                    splane[(k,) + pq][
                                        :, r0 + 8 * s:r0 + 8 * s + 8, :],
                                    start=(k == 0), stop=(k == 1),
                                )
                    with tc.high_priority(offset=30):
                        nc.scalar.activation(
                            vp[:, 32 * half:32 * half + 16, :], ps[:],
                            AF.Identity, bias=bv_t[:])
                        nc.scalar.activation(
                            vp[:, 32 * half + 16:32 * half + 32, :], ps2[:],
                            AF.Identity, bias=bv_t[:])

            for pq in CONV_ORDER:
                conv_plane(pq)

            # fold half0 passes (t-major, WAW-ordered by emission)
            for t_ij in FOLD_ORDER:
                fold_pass(t_ij, 0, stop=(t_ij == FOLD_ORDER[-1]))

        # conv pool closed -> fold half1 banks free
        psfB = ctx.enter_context(
            tc.tile_pool(name="psfB", bufs=1, space="PSUM"))
        fold_h.append(psfB.tile([128, 32, W], F32, tag="foldh1", name="foldh1"))
        for s in range(4):
            nc.tensor.matmul(
                fold_h[1][:, 8 * s:8 * s + 8, :], zero_t[:],
                splane[(0, 0, 0)][:, 32 + 8 * s:32 + 8 * s + 8, :],
                start=True, stop=False, skip_group_check=True,
            )
        for t_ij in FOLD_ORDER:
            fold_pass(t_ij, 1, stop=(t_ij == FOLD_ORDER[-1]))

        # --- normalization + final: out = (fold * rz) * feat, per band ---
        z_t = pool.tile([128, 8], F32, tag="z")
        rz_t = pool.tile([128, 1], F32, tag="rz")
        with tc.high_priority():
            nc.vector.tensor_reduce(z_t[:, 0:1], exp_t[:, 0:9],
                                    axis=mybir.AxisListType.X, op=ALU.add)
            nc.vector.reciprocal(rz_t[:], z_t[:, 0:1])
        out_t = pool.tile([128, H, W], BF16, tag="out")
        scf = [pool.tile([128, 16, W], BF16, tag=f"scf{i}", name=f"scf{i}") for i in range(2)]
        for band in range(4):
            half, sub = divmod(band, 2)
            fview = fold_h[half][:, 16 * sub:16 * sub + 16, :]
            feat_b = fbt[band][:, 0, :, :]
            oview = out_t[:, 16 * band:16 * band + 16, :]
            if half == 0:
                with tc.high_priority(offset=20):
                    nc.scalar.activation(scf[sub][:], fview,
                                         AF.Identity, scale=rz_t[:])
                    nc.vector.tensor_mul(oview, scf[sub][:], feat_b)
            else:
                with tc.high_priority(offset=20):
                    nc.vector.scalar_tensor_tensor(
                        out=oview, in0=fview, scalar=rz_t[:, 0:1],
                        in1=feat_b, op0=ALU.mult, op1=ALU.mult)
            nc.sync.dma_start(out_d.ap()[:, 16 * band:16 * band + 16, :],
                              oview)

    nc.compile()
    nc.m = get_hw_module(nc.m)
    return nc


_PROGRAM = None


def _get_program():
    global _PROGRAM
    if _PROGRAM is None:
        _PROGRAM = build_program()
    return _PROGRAM


def _prep_inputs(feat, src, Wq, bq, Wv, bv):
    bf = ml_dtypes.bfloat16
    # src parity planes: (B, ct, p, q, 128, H, W); ct0 = own half
    spl = np.ascontiguousarray(
        src.reshape(B, 2, 128, H, 2, W, 2).transpose(0, 1, 4, 6, 2, 3, 5)
    ).astype(bf)
    featb = feat.reshape(B, 2, 128, H, W).astype(bf)
    wq3 = np.ascontiguousarray(Wq.T).reshape(2, 128, C)
    wv3 = np.ascontiguousarray(Wv.T).reshape(2, 128, C)
    identb = np.eye(128, dtype=np.float32).astype(bf)
    in_maps = []
    for core in range(N_CORES):
        b, h = divmod(core, 2)
        oc = slice(h * 128, h * 128 + 128)
        order = [h, 1 - h]  # slot 0 = own input-channel half
        # featbands: (4, 2, 128, 16, W) - row-band major, slot0 = own half
        fb = np.ascontiguousarray(
            featb[b][order].reshape(2, 128, 4, 16, W).transpose(2, 0, 1, 3, 4))
        spl_core = np.stack([
            spl[b][order[ct]][p][q] for (p, q), ct in DMA_PLANES
        ])
        in_maps.append(
            dict(
                featbands=fb,
                splanes=np.ascontiguousarray(spl_core),
                wq=np.ascontiguousarray(wq3[order][:, :, oc]).astype(bf),
                wv=np.ascontiguousarray(wv3[order][:, :, oc]).astype(bf),
                bq=bq[oc].reshape(128, 1).astype(np.float32),
                bv=bv[oc].reshape(128, 1).astype(np.float32),
                identb=identb,
            )
        )
    return in_maps


def kernel(feat, src, Wq, bq, Wv, bv, _trace=False):
    feat = np.asarray(feat, np.float32)
    src = np.asarray(src, np.float32)
    Wq = np.asarray(Wq, np.float32)
    bq = np.asarray(bq, np.float32)
    Wv = np.asarray(Wv, np.float32)
    bv = np.asarray(bv, np.float32)

    in_maps = _prep_inputs(feat, src, Wq, bq, Wv, bv)
    nc = _get_program()
    res = bass_utils.run_bass_kernel_spmd(
        nc, in_maps, core_ids=list(range(N_CORES)), trace=_trace
    )
    out = np.empty((B, C, H, W), np.float32)
    for core in range(N_CORES):
        b, h = divmod(core, 2)
        out[b, h * 128:h * 128 + 128] = np.asarray(
            res.results[core]["out"], dtype=np.float32)
    if _trace:
        kernel.last_results = res
    return out


kernel.last_results = None


# revision 8
# speedup vs baseline: 1.0763x; 1.0153x over previous
"""Trainium2 Bass kernel v3 for nn_CRAP_16544214024675 (sparse_attention).

Reference computation (per batch b, channel c):
  q = Wq@feat + bq                        (1x1 conv over channels)
  k = unfold3x3_s2(src)                   (strided window gather, pad 1)
  v = unfold3x3_s2(Wv@src + bv)
  A = softmax_t( sum_px q*k_t / 64 )      (9 window positions)
  out = fold3x3_s1( A_t * v_t ) * feat

Sharding: 8 cores = 4 batches x 2 output-channel halves (slot 0 of packed
inputs = the core's own half; channel sums commute).

v3 vs v2 (79.6us baseline): full reschedule for overlap.
 - DMA: featb as 4 row-band transfers (early q-conv start), then src
   parity planes ct0-first (products are the long pole), ct1 planes after,
   ct1 of plane (0,0) last (its vplane feeds only the late fold passes).
 - products: 7 DVE STT + 2 gpsimd mul (plane-(1,1) t's) with ACT reduces.
   No shifted-q copy (STT runs 1x, no alignment constraint).
 - fold in two half-image PSUM phases (4 banks each): half0 coexists with
   the conv chunk pool (2x2 banks) and accumulates t-by-t as exps arrive;
   half1 reuses the conv banks after convs drain. Zero-prefill per half
   keeps accumulation order free.
 - v-convs K-split: ct0 matmuls run as soon as the ct0 plane lands, ct1
   matmuls chase the late DMA.
 - final out = (fold*rz)*feat per 16-row band: bands 0-1 via ACT scale +
   DVE mul, bands 2-3 via one DVE STT each; band DMA-out chases.
"""
import sys
from contextlib import ExitStack

import numpy as np

for _p in ("/opt/trn_rl_repo", "/root/.axon_site/_ro/trn_rl_repo"):
    if _p not in sys.path:
        sys.path.append(_p)

import ml_dtypes

import concourse.tile as tile
from concourse import bacc, mybir
from concourse import bass_utils
from concourse.bass_interp import get_hw_module

F32 = mybir.dt.float32
BF16 = mybir.dt.bfloat16
AF = mybir.ActivationFunctionType
ALU = mybir.AluOpType

B, C, H, W = 4, 256, 64, 64
N_CORES = 8

# src parity plane (p,q) -> t's whose product reads its ct0 and whose fold
# reads vplane (p,q)
PLANE_TS = {
    (0, 0): [(1, 1)],
    (1, 1): [(2, 2), (0, 0), (0, 2), (2, 0)],
    (1, 0): [(0, 1), (2, 1)],
    (0, 1): [(1, 0), (1, 2)],
}
# DMA arrival order of (plane, ct): all ct0 first (product chain), then
# ct1; plane (0,0) ct1 last (only feeds the final fold passes), split in 2.
DMA_PLANES = [
    ((0, 0), 0), ((1, 1), 0), ((1, 1), 1), ((1, 0), 0),
    ((1, 0), 1), ((0, 1), 0), ((0, 1), 1), ((0, 0), 1),
]
CONV_ORDER = [(1, 1), (1, 0), (0, 1), (0, 0)]
# per-t product engine: 'S' DVE STT fused; 'A' DVE 2x mul + ACT reduce
# (A-type t's have j in {1,2} so both product views stay alignable)
T_TYPE = {
    (1, 1): 'S', (2, 2): 'A', (0, 0): 'S', (0, 2): 'S', (2, 0): 'S',
    (0, 1): 'A', (2, 1): 'A', (1, 0): 'S', (1, 2): 'S',
}
# product emission order (DVE/GPS priority); follows ct0 arrival
T_ORDER = [(1, 1), (2, 2), (0, 0), (2, 0), (0, 2),
           (0, 1), (2, 1), (1, 0), (1, 2)]
T_IDX = {t: i for i, t in enumerate(T_ORDER)}
# fold emission order per half = expected readiness order (WAW serialized)
FOLD_ORDER = [(2, 2), (0, 0), (0, 2), (2, 0), (0, 1),
              (2, 1), (1, 0), (1, 2), (1, 1)]
N_WARM = 40
N_FILL = 40


def _prod_views(i, j):
    qr0, pr0, rows = (1, 0, 63) if i == 0 else (0, 0, 64)
    qc0, pc0, cols = (1, 0, 63) if j == 0 else (0, 0, 64)
    return qr0, pr0, rows, qc0, pc0, cols


def _windows(i, j):
    if i == 0:
        yo0, yo1, dy = 0, 63, 0
    elif i == 1:
        yo0, yo1, dy = 0, 64, 0
    else:
        yo0, yo1, dy = 1, 64, -1
    if j == 0:
        xo0, xo1, dx = 0, 63, 0
    elif j == 1:
        xo0, xo1, dx = 0, 64, 0
    else:
        xo0, xo1, dx = 1, 64, -1
    return yo0, yo1, dy, xo0, xo1, dx


def build_program():
    nc = bacc.Bacc("TRN2", target_bir_lowering=False, debug=False)

    fb_d = nc.dram_tensor("featbands", (4, 2, 128, 16, W), BF16,
                          kind="ExternalInput")
    spl_d = nc.dram_tensor("splanes", (8, 128, H, W), BF16,
                           kind="ExternalInput")
    wq_d = nc.dram_tensor("wq", (2, 128, 128), BF16, kind="ExternalInput")
    wv_d = nc.dram_tensor("wv", (2, 128, 128), BF16, kind="ExternalInput")
    bq_d = nc.dram_tensor("bq", (128, 1), F32, kind="ExternalInput")
    bv_d = nc.dram_tensor("bv", (128, 1), F32, kind="ExternalInput")
    id_d = nc.dram_tensor("identb", (128, 128), BF16, kind="ExternalInput")
    out_d = nc.dram_tensor("out", (128, H, W), BF16, kind="ExternalOutput")

    with tile.TileContext(nc) as tc, ExitStack() as ctx:
        pool = ctx.enter_context(tc.tile_pool(name="main", bufs=1))
        scpool = ctx.enter_context(tc.tile_pool(name="scratch", bufs=3))
        wtpool = ctx.enter_context(tc.tile_pool(name="wts", bufs=9))

        # --- input DMA: emission order = priority order ---
        wq_t = pool.tile([128, 2, 128], BF16, tag="wq")
        wv_t = pool.tile([128, 2, 128], BF16, tag="wv")
        bq_t = pool.tile([128, 1], F32, tag="bq")
        bv_t = pool.tile([128, 1], F32, tag="bv")
        nc.sync.dma_start(wq_t[:], wq_d.ap().rearrange("a p b -> p a b"))
        nc.sync.dma_start(bq_t[:], bq_d.ap())
        fbt = []
        for b in range(4):
            t_ = pool.tile([128, 2, 16, W], BF16, tag=f"fb{b}")
            nc.sync.dma_start(t_[:], fb_d.ap()[b].rearrange("h p r c -> p h r c"))
            fbt.append(t_)
        nc.sync.dma_start(wv_t[:], wv_d.ap().rearrange("a p b -> p a b"))
        nc.sync.dma_start(bv_t[:], bv_d.ap())
        id_t = pool.tile([128, 128], BF16, tag="identb")
        nc.sync.dma_start(id_t[:], id_d.ap())
        splane = {}
        for si, (pq, ct) in enumerate(DMA_PLANES):
            t_ = pool.tile([128, H, W], BF16, tag=f"spl{ct}{pq[0]}{pq[1]}",
                           name=f"spl{ct}{pq[0]}{pq[1]}")
            splane[(ct,) + pq] = t_
            if si == len(DMA_PLANES) - 1:
                nc.sync.dma_start(t_[:, 0:32, :], spl_d.ap()[si, :, 0:32, :])
                nc.sync.dma_start(t_[:, 32:64, :], spl_d.ap()[si, :, 32:64, :])
            else:
                nc.sync.dma_start(t_[:], spl_d.ap()[si])

        lg_t = pool.tile([128, 16], F32, tag="lg")
        exp_t = pool.tile([128, 16], F32, tag="exp")
        q_t = pool.tile([128, H, W], BF16, tag="q")
        zero_t = pool.tile([128, 128], BF16, tag="zero")
        nc.gpsimd.memset(zero_t[:], 0.0)
        warm_t = pool.tile([128, 256], BF16, tag="warm")
        nc.gpsimd.memset(warm_t[:], 0.5)

        # trigger the ACT exp-table load immediately (off the critical path)
        nc.gpsimd.memset(lg_t[:, 15:16], 0.0)
        with tc.high_priority():
            nc.scalar.activation(exp_t[:, 15:16], lg_t[:, 15:16], AF.Exp,
                                 scale=1.0 / 64.0)

        # PSUM: conv chunks 2x[128,16,W] (4 banks) + fold half0 (4 banks);
        # fold half1 opens after the conv pool closes.
        psc_cm = tc.tile_pool(name="psc", bufs=2, space="PSUM")
        psfA = ctx.enter_context(
            tc.tile_pool(name="psfA", bufs=1, space="PSUM"))
        fold_h = [psfA.tile([128, 32, W], F32, tag="foldh0", name="foldh0")]

        def fold_prefill(fps):
            for s in range(4):
                nc.tensor.matmul(
                    fps[:, 8 * s:8 * s + 8, :], zero_t[:],
                    splane[(0, 0, 0)][:, 8 * s:8 * s + 8, :],
                    start=True, stop=False, skip_group_check=True,
                )

        def fold_pass(t_ij, half, stop):
            i, j = t_ij
            yo0, yo1, dy, xo0, xo1, dx = _windows(i, j)
            pq = ((i + 1) % 2, (j + 1) % 2)
            vp = vplane[pq]
            dg = diags[(i, j)]
            fps = fold_h[half]
            r0 = 32 * half
            yb = max(yo0, r0)
            yend = min(yo1, r0 + 32)
            while yb < yend:
                ye = min((yb // 8 + 1) * 8, yend)
                nc.tensor.matmul(
                    fps[:, yb - r0:ye - r0, xo0:xo1],
                    dg[:],
                    vp[:, yb + dy:ye + dy, xo0 + dx:xo1 + dx],
                    start=False, stop=stop,
                    skip_group_check=True,
                )
                yb = ye

        vplane = {}
        diags = {}
        with psc_cm as psc:
            # PE warm-up burst (dependency-free) to ramp the clock
            wps = psc.tile([128, 16, W], F32, tag="cv", name="warmps")
            for w_i in range(N_WARM):
                nc.tensor.matmul(
                    wps[:, 0:2, :], warm_t[:, 0:128], warm_t[:, 128:256],
                    start=True, stop=True, skip_group_check=True,
                )

            # q-conv: 4 row-band chunks chasing the featb DMAs
            for b in range(4):
                ps = psc.tile([128, 16, W], F32, tag="cv", name=f"qps{b}")
                for s in range(2):
                    for k in range(2):
                        nc.tensor.matmul(
                            ps[:, 8 * s:8 * s + 8, :],
                            wq_t[:, k, :],
                            fbt[b][:, k, 8 * s:8 * s + 8, :],
                            start=(k == 0), stop=(k == 1),
                        )
                with tc.high_priority(offset=60):
                    nc.scalar.activation(q_t[:, 16 * b:16 * b + 16, :],
                                         ps[:], AF.Identity, bias=bq_t[:])

            # fold half0 prefill early (PE is otherwise waiting on DMA)
            fold_prefill(fold_h[0])
            # filler burst: bridge the PE gap between q-conv and the first
            # v-conv so the clock stays ramped
            fps_ = psc.tile([128, 16, W], F32, tag="cv", name="fillps")
            for _f in range(N_FILL):
                nc.tensor.matmul(
                    fps_[:, 0:2, :], warm_t[:, 0:128], warm_t[:, 128:256],
                    start=True, stop=True, skip_group_check=True,
                )

            # --- products ---
            def product(t_ij):
                i, j = t_ij
                idx = T_IDX[t_ij]
                pq = [k for k, ts in PLANE_TS.items() if t_ij in ts][0]
                pl = splane[(0,) + pq]
                qr0, pr0, rows, qc0, pc0, cols = _prod_views(i, j)
                q_view = q_t[:, qr0:qr0 + rows, qc0:qc0 + cols]
                p_view = pl[:, pr0:pr0 + rows, pc0:pc0 + cols]
                sc = scpool.tile([128, H, W], BF16, tag="prod",
                                 name=f"prod{idx}")
                if T_TYPE[t_ij] == 'A':
                    with tc.high_priority(offset=50):
                        nc.vector.tensor_mul(sc[:, 0:rows, 0:cols],
                                             q_view, p_view)
                    with tc.high_priority(offset=50):
                        nc.scalar.activation(
                            sc[:, 0:rows, 0:cols], sc[:, 0:rows, 0:cols],
                            AF.Copy, accum_out=lg_t[:, idx:idx + 1])
                else:
                    with tc.high_priority(offset=50):
                        nc.vector.scalar_tensor_tensor(
                            out=sc[:, 0:rows, 0:cols], in0=q_view,
                            scalar=1.0, in1=p_view,
                            op0=ALU.mult, op1=ALU.mult,
                            accum_out=lg_t[:, idx:idx + 1])
                with tc.high_priority():
                    nc.scalar.activation(
                        exp_t[:, idx:idx + 1], lg_t[:, idx:idx + 1],
                        AF.Exp, scale=1.0 / 64.0)
                dg = wtpool.tile([128, 128], BF16, tag="wt",
                                 name=f"dg{idx}")
                diags[t_ij] = dg
                with tc.high_priority():
                    nc.vector.tensor_scalar(
                        out=dg[:], in0=id_t[:],
                        scalar1=exp_t[:, idx:idx + 1], scalar2=None,
                        op0=ALU.mult)

            for t_ij in T_ORDER:
                product(t_ij)

            # --- v-convs (K-split: ct0 part early, ct1 part chases DMA) ---
            def conv_plane(pq):
                vp = pool.tile([128, H, W], BF16, tag=f"vpl{pq[0]}{pq[1]}",
                               name=f"vpl{pq[0]}{pq[1]}")
                vplane[pq] = vp
                for half in range(2):
                    ps = psc.tile([128, 16, W], F32, tag="cv",
                                  name=f"cv{pq[0]}{pq[1]}h{half}a")
                    ps2 = psc.tile([128, 16, W], F32, tag="cv",
                                   name=f"cv{pq[0]}{pq[1]}h{half}b")
                    for ci, ps_ in enumerate((ps, ps2)):
                        r0 = 32 * half + 16 * ci
                        for s in range(2):
                            for k in range(2):
                                nc.tensor.matmul(
                                    ps_[:, 8 * s:8 * s + 8, :],
                                    wv_t[:, k, :],
                                    splane[(k,) + pq][
                                        :, r0 + 8 * s:r0 + 8 * s + 8, :],
                                    start=(k == 0), stop=(k == 1),
                                )
                    with tc.high_priority(offset=30):
                        nc.scalar.activation(
                            vp[:, 32 * half:32 * half + 16, :], ps[:],
                            AF.Identity, bias=bv_t[:])
                        nc.scalar.activation(
                            vp[:, 32 * half + 16:32 * half + 32, :], ps2[:],
                            AF.Identity, bias=bv_t[:])

            for pq in CONV_ORDER:
                conv_plane(pq)


        # conv pool closed -> fold half1 banks free
        psfB = ctx.enter_context(
            tc.tile_pool(name="psfB", bufs=1, space="PSUM"))
        fold_h.append(psfB.tile([128, 32, W], F32, tag="foldh1", name="foldh1"))
        for s in range(4):
            nc.tensor.matmul(
                fold_h[1][:, 8 * s:8 * s + 8, :], zero_t[:],
                splane[(0, 0, 0)][:, 32 + 8 * s:32 + 8 * s + 8, :],
                start=True, stop=False, skip_group_check=True,
            )
        for t_ij in FOLD_ORDER:
            fold_pass(t_ij, 0, stop=(t_ij == FOLD_ORDER[-1]))
            fold_pass(t_ij, 1, stop=(t_ij == FOLD_ORDER[-1]))

        # --- normalization + final: out = (fold * rz) * feat, per band ---
        z_t = pool.tile([128, 8], F32, tag="z")
        rz_t = pool.tile([128, 1], F32, tag="rz")
        with tc.high_priority():
            nc.vector.tensor_reduce(z_t[:, 0:1], exp_t[:, 0:9],
                                    axis=mybir.AxisListType.X, op=ALU.add)
            nc.vector.reciprocal(rz_t[:], z_t[:, 0:1])
        out_t = pool.tile([128, H, W], BF16, tag="out")
        scf = [pool.tile([128, 16, W], BF16, tag=f"scf{i}", name=f"scf{i}") for i in range(2)]
        for band in range(4):
            half, sub = divmod(band, 2)
            fview = fold_h[half][:, 16 * sub:16 * sub + 16, :]
            feat_b = fbt[band][:, 0, :, :]
            oview = out_t[:, 16 * band:16 * band + 16, :]
            if half == 0:
                with tc.high_priority(offset=20):
                    nc.scalar.activation(scf[sub][:], fview,
                                         AF.Identity, scale=rz_t[:])
                    nc.vector.tensor_mul(oview, scf[sub][:], feat_b)
            else:
                with tc.high_priority(offset=20):
                    nc.vector.scalar_tensor_tensor(
                        out=oview, in0=fview, scalar=rz_t[:, 0:1],
                        in1=feat_b, op0=ALU.mult, op1=ALU.mult)
            nc.sync.dma_start(out_d.ap()[:, 16 * band:16 * band + 16, :],
                              oview)

    nc.compile()
    nc.m = get_hw_module(nc.m)
    return nc


_PROGRAM = None


def _get_program():
    global _PROGRAM
    if _PROGRAM is None:
        _PROGRAM = build_program()
    return _PROGRAM


def _prep_inputs(feat, src, Wq, bq, Wv, bv):
    bf = ml_dtypes.bfloat16
    # src parity planes: (B, ct, p, q, 128, H, W); ct0 = own half
    spl = np.ascontiguousarray(
        src.reshape(B, 2, 128, H, 2, W, 2).transpose(0, 1, 4, 6, 2, 3, 5)
    ).astype(bf)
    featb = feat.reshape(B, 2, 128, H, W).astype(bf)
    wq3 = np.ascontiguousarray(Wq.T).reshape(2, 128, C)
    wv3 = np.ascontiguousarray(Wv.T).reshape(2, 128, C)
    identb = np.eye(128, dtype=np.float32).astype(bf)
    in_maps = []
    for core in range(N_CORES):
        b, h = divmod(core, 2)
        oc = slice(h * 128, h * 128 + 128)
        order = [h, 1 - h]  # slot 0 = own input-channel half
        # featbands: (4, 2, 128, 16, W) - row-band major, slot0 = own half
        fb = np.ascontiguousarray(
            featb[b][order].reshape(2, 128, 4, 16, W).transpose(2, 0, 1, 3, 4))
        spl_core = np.stack([
            spl[b][order[ct]][p][q] for (p, q), ct in DMA_PLANES
        ])
        in_maps.append(
            dict(
                featbands=fb,
                splanes=np.ascontiguousarray(spl_core),
                wq=np.ascontiguousarray(wq3[order][:, :, oc]).astype(bf),
                wv=np.ascontiguousarray(wv3[order][:, :, oc]).astype(bf),
                bq=bq[oc].reshape(128, 1).astype(np.float32),
                bv=bv[oc].reshape(128, 1).astype(np.float32),
                identb=identb,
            )
        )
    return in_maps


def kernel(feat, src, Wq, bq, Wv, bv, _trace=False):
    feat = np.asarray(feat, np.float32)
    src = np.asarray(src, np.float32)
    Wq = np.asarray(Wq, np.float32)
    bq = np.asarray(bq, np.float32)
    Wv = np.asarray(Wv, np.float32)
    bv = np.asarray(bv, np.float32)

    in_maps = _prep_inputs(feat, src, Wq, bq, Wv, bv)
    nc = _get_program()
    res = bass_utils.run_bass_kernel_spmd(
        nc, in_maps, core_ids=list(range(N_CORES)), trace=_trace
    )
    out = np.empty((B, C, H, W), np.float32)
    for core in range(N_CORES):
        b, h = divmod(core, 2)
        out[b, h * 128:h * 128 + 128] = np.asarray(
            res.results[core]["out"], dtype=np.float32)
    if _trace:
        kernel.last_results = res
    return out


kernel.last_results = None


# revision 9
# speedup vs baseline: 1.0935x; 1.0160x over previous
"""Trainium2 Bass kernel v3 for nn_CRAP_16544214024675 (sparse_attention).

Reference computation (per batch b, channel c):
  q = Wq@feat + bq                        (1x1 conv over channels)
  k = unfold3x3_s2(src)                   (strided window gather, pad 1)
  v = unfold3x3_s2(Wv@src + bv)
  A = softmax_t( sum_px q*k_t / 64 )      (9 window positions)
  out = fold3x3_s1( A_t * v_t ) * feat

Sharding: 8 cores = 4 batches x 2 output-channel halves (slot 0 of packed
inputs = the core's own half; channel sums commute).

v3 vs v2 (79.6us baseline): full reschedule for overlap.
 - DMA: featb as 4 row-band transfers (early q-conv start), then src
   parity planes ct0-first (products are the long pole), ct1 planes after,
   ct1 of plane (0,0) last (its vplane feeds only the late fold passes).
 - products: 7 DVE STT + 2 gpsimd mul (plane-(1,1) t's) with ACT reduces.
   No shifted-q copy (STT runs 1x, no alignment constraint).
 - fold in two half-image PSUM phases (4 banks each): half0 coexists with
   the conv chunk pool (2x2 banks) and accumulates t-by-t as exps arrive;
   half1 reuses the conv banks after convs drain. Zero-prefill per half
   keeps accumulation order free.
 - v-convs K-split: ct0 matmuls run as soon as the ct0 plane lands, ct1
   matmuls chase the late DMA.
 - final out = (fold*rz)*feat per 16-row band: bands 0-1 via ACT scale +
   DVE mul, bands 2-3 via one DVE STT each; band DMA-out chases.
"""
import sys
from contextlib import ExitStack

import numpy as np

for _p in ("/opt/trn_rl_repo", "/root/.axon_site/_ro/trn_rl_repo"):
    if _p not in sys.path:
        sys.path.append(_p)

import ml_dtypes

import concourse.tile as tile
from concourse import bacc, mybir
from concourse import bass_utils
from concourse.bass_interp import get_hw_module

F32 = mybir.dt.float32
BF16 = mybir.dt.bfloat16
AF = mybir.ActivationFunctionType
ALU = mybir.AluOpType

B, C, H, W = 4, 256, 64, 64
N_CORES = 8

# src parity plane (p,q) -> t's whose product reads its ct0 and whose fold
# reads vplane (p,q)
PLANE_TS = {
    (0, 0): [(1, 1)],
    (1, 1): [(2, 2), (0, 0), (0, 2), (2, 0)],
    (1, 0): [(0, 1), (2, 1)],
    (0, 1): [(1, 0), (1, 2)],
}
# DMA arrival order of (plane, ct): all ct0 first (product chain), then
# ct1; plane (0,0) ct1 last (only feeds the final fold passes), split in 2.
DMA_PLANES = [
    ((0, 0), 0), ((0, 0), 1), ((1, 1), 0), ((1, 1), 1),
    ((1, 0), 1), ((0, 1), 1), ((1, 0), 0), ((0, 1), 0),
]
CONV_ORDER = [(0, 0), (1, 1), (1, 0), (0, 1)]
# per-t product engine: 'S' DVE STT fused; 'A' DVE 2x mul + ACT reduce
# (A-type t's have j in {1,2} so both product views stay alignable)
T_TYPE = {
    (1, 1): 'S', (2, 2): 'A', (0, 0): 'S', (0, 2): 'S', (2, 0): 'S',
    (0, 1): 'A', (2, 1): 'A', (1, 0): 'S', (1, 2): 'S',
}
# product emission order (DVE/GPS priority); follows ct0 arrival
T_ORDER = [(1, 1), (2, 2), (0, 0), (2, 0), (0, 2),
           (0, 1), (2, 1), (1, 0), (1, 2)]
T_IDX = {t: i for i, t in enumerate(T_ORDER)}
# fold emission order per half = expected readiness order (WAW serialized)
FOLD_ORDER = [(1, 1), (2, 2), (0, 0), (0, 2), (2, 0),
              (0, 1), (2, 1), (1, 0), (1, 2)]
N_WARM = 55
N_FILL = 30


def _prod_views(i, j):
    qr0, pr0, rows = (1, 0, 63) if i == 0 else (0, 0, 64)
    qc0, pc0, cols = (1, 0, 63) if j == 0 else (0, 0, 64)
    return qr0, pr0, rows, qc0, pc0, cols


def _windows(i, j):
    if i == 0:
        yo0, yo1, dy = 0, 63, 0
    elif i == 1:
        yo0, yo1, dy = 0, 64, 0
    else:
        yo0, yo1, dy = 1, 64, -1
    if j == 0:
        xo0, xo1, dx = 0, 63, 0
    elif j == 1:
        xo0, xo1, dx = 0, 64, 0
    else:
        xo0, xo1, dx = 1, 64, -1
    return yo0, yo1, dy, xo0, xo1, dx


def build_program():
    nc = bacc.Bacc("TRN2", target_bir_lowering=False, debug=False)

    fb_d = nc.dram_tensor("featbands", (4, 2, 128, 16, W), BF16,
                          kind="ExternalInput")
    spl_d = nc.dram_tensor("splanes", (8, 128, H, W), BF16,
                           kind="ExternalInput")
    wq_d = nc.dram_tensor("wq", (2, 128, 128), BF16, kind="ExternalInput")
    wv_d = nc.dram_tensor("wv", (2, 128, 128), BF16, kind="ExternalInput")
    bq_d = nc.dram_tensor("bq", (128, 1), F32, kind="ExternalInput")
    bv_d = nc.dram_tensor("bv", (128, 1), F32, kind="ExternalInput")
    id_d = nc.dram_tensor("identb", (128, 128), BF16, kind="ExternalInput")
    out_d = nc.dram_tensor("out", (128, H, W), BF16, kind="ExternalOutput")

    with tile.TileContext(nc) as tc, ExitStack() as ctx:
        pool = ctx.enter_context(tc.tile_pool(name="main", bufs=1))
        scpool = ctx.enter_context(tc.tile_pool(name="scratch", bufs=3))
        wtpool = ctx.enter_context(tc.tile_pool(name="wts", bufs=9))

        # --- input DMA: emission order = priority order ---
        wq_t = pool.tile([128, 2, 128], BF16, tag="wq")
        wv_t = pool.tile([128, 2, 128], BF16, tag="wv")
        bq_t = pool.tile([128, 1], F32, tag="bq")
        bv_t = pool.tile([128, 1], F32, tag="bv")
        nc.sync.dma_start(wq_t[:], wq_d.ap().rearrange("a p b -> p a b"))
        nc.sync.dma_start(bq_t[:], bq_d.ap())
        fbt = []
        for b in range(4):
            t_ = pool.tile([128, 2, 16, W], BF16, tag=f"fb{b}")
            nc.sync.dma_start(t_[:], fb_d.ap()[b].rearrange("h p r c -> p h r c"))
            fbt.append(t_)
        nc.sync.dma_start(wv_t[:], wv_d.ap().rearrange("a p b -> p a b"))
        nc.sync.dma_start(bv_t[:], bv_d.ap())
        id_t = pool.tile([128, 128], BF16, tag="identb")
        nc.sync.dma_start(id_t[:], id_d.ap())
        splane = {}
        for si, (pq, ct) in enumerate(DMA_PLANES):
            t_ = pool.tile([128, H, W], BF16, tag=f"spl{ct}{pq[0]}{pq[1]}",
                           name=f"spl{ct}{pq[0]}{pq[1]}")
            splane[(ct,) + pq] = t_
            nc.sync.dma_start(t_[:], spl_d.ap()[si])

        lg_t = pool.tile([128, 16], F32, tag="lg")
        exp_t = pool.tile([128, 16], F32, tag="exp")
        q_t = pool.tile([128, H, W], BF16, tag="q")
        zero_t = pool.tile([128, 128], BF16, tag="zero")
        nc.gpsimd.memset(zero_t[:], 0.0)
        warm_t = pool.tile([128, 256], BF16, tag="warm")
        nc.gpsimd.memset(warm_t[:], 0.5)

        # trigger the ACT exp-table load immediately (off the critical path)
        nc.gpsimd.memset(lg_t[:, 15:16], 0.0)
        with tc.high_priority():
            nc.scalar.activation(exp_t[:, 15:16], lg_t[:, 15:16], AF.Exp,
                                 scale=1.0 / 64.0)

        # PSUM: conv chunks 2x[128,16,W] (4 banks) + fold half0 (4 banks);
        # fold half1 opens after the conv pool closes.
        psc_cm = tc.tile_pool(name="psc", bufs=2, space="PSUM")
        psfA = ctx.enter_context(
            tc.tile_pool(name="psfA", bufs=1, space="PSUM"))
        fold_h = [psfA.tile([128, 32, W], F32, tag="foldh0", name="foldh0")]

        def fold_pass(t_ij, half, stop):
            i, j = t_ij
            yo0, yo1, dy, xo0, xo1, dx = _windows(i, j)
            pq = ((i + 1) % 2, (j + 1) % 2)
            vp = vplane[pq]
            dg = diags[(i, j)]
            fps = fold_h[half]
            r0 = 32 * half
            yb = max(yo0, r0)
            yend = min(yo1, r0 + 32)
            while yb < yend:
                ye = min((yb // 8 + 1) * 8, yend)
                nc.tensor.matmul(
                    fps[:, yb - r0:ye - r0, xo0:xo1],
                    dg[:],
                    vp[:, yb + dy:ye + dy, xo0 + dx:xo1 + dx],
                    start=(t_ij == (1, 1)), stop=stop,
                    skip_group_check=True,
                )
                yb = ye

        vplane = {}
        diags = {}
        with psc_cm as psc:
            # PE warm-up burst (dependency-free) to ramp the clock
            wps = psc.tile([128, 16, W], F32, tag="cv", name="warmps")
            for w_i in range(N_WARM):
                nc.tensor.matmul(
                    wps[:, 0:2, :], warm_t[:, 0:128], warm_t[:, 128:256],
                    start=True, stop=True, skip_group_check=True,
                )

            # q-conv: 4 row-band chunks chasing the featb DMAs
            for b in range(4):
                ps = psc.tile([128, 16, W], F32, tag="cv", name=f"qps{b}")
                for s in range(2):
                    for k in range(2):
                        nc.tensor.matmul(
                            ps[:, 8 * s:8 * s + 8, :],
                            wq_t[:, k, :],
                            fbt[b][:, k, 8 * s:8 * s + 8, :],
                            start=(k == 0), stop=(k == 1),
                        )
                with tc.high_priority(offset=60):
                    nc.scalar.activation(q_t[:, 16 * b:16 * b + 16, :],
                                         ps[:], AF.Identity, bias=bq_t[:])


            # --- products ---
            def product(t_ij):
                i, j = t_ij
                idx = T_IDX[t_ij]
                pq = [k for k, ts in PLANE_TS.items() if t_ij in ts][0]
                pl = splane[(0,) + pq]
                qr0, pr0, rows, qc0, pc0, cols = _prod_views(i, j)
                q_view = q_t[:, qr0:qr0 + rows, qc0:qc0 + cols]
                p_view = pl[:, pr0:pr0 + rows, pc0:pc0 + cols]
                sc = scpool.tile([128, H, W], BF16, tag="prod",
                                 name=f"prod{idx}")
                if T_TYPE[t_ij] == 'A':
                    with tc.high_priority(offset=50):
                        nc.vector.tensor_mul(sc[:, 0:rows, 0:cols],
                                             q_view, p_view)
                    with tc.high_priority(offset=50):
                        nc.scalar.activation(
                            sc[:, 0:rows, 0:cols], sc[:, 0:rows, 0:cols],
                            AF.Copy, accum_out=lg_t[:, idx:idx + 1])
                else:
                    with tc.high_priority(offset=50):
                        nc.vector.scalar_tensor_tensor(
                            out=sc[:, 0:rows, 0:cols], in0=q_view,
                            scalar=1.0, in1=p_view,
                            op0=ALU.mult, op1=ALU.mult,
                            accum_out=lg_t[:, idx:idx + 1])
                with tc.high_priority():
                    nc.scalar.activation(
                        exp_t[:, idx:idx + 1], lg_t[:, idx:idx + 1],
                        AF.Exp, scale=1.0 / 64.0)
                dg = wtpool.tile([128, 128], BF16, tag="wt",
                                 name=f"dg{idx}")
                diags[t_ij] = dg
                with tc.high_priority():
                    nc.vector.tensor_scalar(
                        out=dg[:], in0=id_t[:],
                        scalar1=exp_t[:, idx:idx + 1], scalar2=None,
                        op0=ALU.mult)

            for t_ij in T_ORDER:
                product(t_ij)

            # --- v-convs (K-split: ct0 part early, ct1 part chases DMA) ---
            def conv_plane(pq):
                vp = pool.tile([128, H, W], BF16, tag=f"vpl{pq[0]}{pq[1]}",
                               name=f"vpl{pq[0]}{pq[1]}")
                vplane[pq] = vp
                for half in range(2):
                    ps = psc.tile([128, 16, W], F32, tag="cv",
                                  name=f"cv{pq[0]}{pq[1]}h{half}a")
                    ps2 = psc.tile([128, 16, W], F32, tag="cv",
                                   name=f"cv{pq[0]}{pq[1]}h{half}b")
                    for ci, ps_ in enumerate((ps, ps2)):
                        r0 = 32 * half + 16 * ci
                        for s in range(2):
                            for k in range(2):
                                nc.tensor.matmul(
                                    ps_[:, 8 * s:8 * s + 8, :],
                                    wv_t[:, k, :],
                                    splane[(k,) + pq][
                                        :, r0 + 8 * s:r0 + 8 * s + 8, :],
                                    start=(k == 0), stop=(k == 1),
                                )
                    with tc.high_priority(offset=30):
                        nc.scalar.activation(
                            vp[:, 32 * half:32 * half + 16, :], ps[:],
                            AF.Identity, bias=bv_t[:])
                        nc.scalar.activation(
                            vp[:, 32 * half + 16:32 * half + 32, :], ps2[:],
                            AF.Identity, bias=bv_t[:])

            for ci, pq in enumerate(CONV_ORDER):
                conv_plane(pq)
                if ci == 1:
                    fps_ = psc.tile([128, 16, W], F32, tag="cv",
                                    name="fillps")
                    for _f in range(N_FILL):
                        nc.tensor.matmul(
                            fps_[:, 0:2, :], warm_t[:, 0:128],
                            warm_t[:, 128:256],
                            start=True, stop=True, skip_group_check=True,
                        )


        # conv pool closed -> fold half1 banks free
        psfB = ctx.enter_context(
            tc.tile_pool(name="psfB", bufs=1, space="PSUM"))
        fold_h.append(psfB.tile([128, 32, W], F32, tag="foldh1", name="foldh1"))
        for t_ij in FOLD_ORDER:
            fold_pass(t_ij, 0, stop=(t_ij == FOLD_ORDER[-1]))
            fold_pass(t_ij, 1, stop=(t_ij == FOLD_ORDER[-1]))

        # --- normalization + final: out = (fold * rz) * feat, per band ---
        z_t = pool.tile([128, 8], F32, tag="z")
        rz_t = pool.tile([128, 1], F32, tag="rz")
        with tc.high_priority():
            nc.vector.tensor_reduce(z_t[:, 0:1], exp_t[:, 0:9],
                                    axis=mybir.AxisListType.X, op=ALU.add)
            nc.vector.reciprocal(rz_t[:], z_t[:, 0:1])
        out_t = pool.tile([128, H, W], BF16, tag="out")
        scf = [pool.tile([128, 16, W], BF16, tag=f"scf{i}", name=f"scf{i}") for i in range(2)]
        for band in range(4):
            half, sub = divmod(band, 2)
            fview = fold_h[half][:, 16 * sub:16 * sub + 16, :]
            feat_b = fbt[band][:, 0, :, :]
            oview = out_t[:, 16 * band:16 * band + 16, :]
            if half == 0:
                with tc.high_priority(offset=20):
                    nc.scalar.activation(scf[sub][:], fview,
                                         AF.Identity, scale=rz_t[:])
                    nc.vector.tensor_mul(oview, scf[sub][:], feat_b)
            else:
                with tc.high_priority(offset=20):
                    nc.vector.scalar_tensor_tensor(
                        out=oview, in0=fview, scalar=rz_t[:, 0:1],
                        in1=feat_b, op0=ALU.mult, op1=ALU.mult)
            nc.sync.dma_start(out_d.ap()[:, 16 * band:16 * band + 16, :],
                              oview)

    nc.compile()
    nc.m = get_hw_module(nc.m)
    return nc


_PROGRAM = None


def _get_program():
    global _PROGRAM
    if _PROGRAM is None:
        _PROGRAM = build_program()
    return _PROGRAM


def _prep_inputs(feat, src, Wq, bq, Wv, bv):
    bf = ml_dtypes.bfloat16
    # src parity planes: (B, ct, p, q, 128, H, W); ct0 = own half
    spl = np.ascontiguousarray(
        src.reshape(B, 2, 128, H, 2, W, 2).transpose(0, 1, 4, 6, 2, 3, 5)
    ).astype(bf)
    featb = feat.reshape(B, 2, 128, H, W).astype(bf)
    wq3 = np.ascontiguousarray(Wq.T).reshape(2, 128, C)
    wv3 = np.ascontiguousarray(Wv.T).reshape(2, 128, C)
    identb = np.eye(128, dtype=np.float32).astype(bf)
    in_maps = []
    for core in range(N_CORES):
        b, h = divmod(core, 2)
        oc = slice(h * 128, h * 128 + 128)
        order = [h, 1 - h]  # slot 0 = own input-channel half
        # featbands: (4, 2, 128, 16, W) - row-band major, slot0 = own half
        fb = np.ascontiguousarray(
            featb[b][order].reshape(2, 128, 4, 16, W).transpose(2, 0, 1, 3, 4))
        spl_core = np.stack([
            spl[b][order[ct]][p][q] for (p, q), ct in DMA_PLANES
        ])
        in_maps.append(
            dict(
                featbands=fb,
                splanes=np.ascontiguousarray(spl_core),
                wq=np.ascontiguousarray(wq3[order][:, :, oc]).astype(bf),
                wv=np.ascontiguousarray(wv3[order][:, :, oc]).astype(bf),
                bq=bq[oc].reshape(128, 1).astype(np.float32),
                bv=bv[oc].reshape(128, 1).astype(np.float32),
                identb=identb,
            )
        )
    return in_maps


def kernel(feat, src, Wq, bq, Wv, bv, _trace=False):
    feat = np.asarray(feat, np.float32)
    src = np.asarray(src, np.float32)
    Wq = np.asarray(Wq, np.float32)
    bq = np.asarray(bq, np.float32)
    Wv = np.asarray(Wv, np.float32)
    bv = np.asarray(bv, np.float32)

    in_maps = _prep_inputs(feat, src, Wq, bq, Wv, bv)
    nc = _get_program()
    res = bass_utils.run_bass_kernel_spmd(
        nc, in_maps, core_ids=list(range(N_CORES)), trace=_trace
    )
    out = np.empty((B, C, H, W), np.float32)
    for core in range(N_CORES):
        b, h = divmod(core, 2)
        out[b, h * 128:h * 128 + 128] = np.asarray(
            res.results[core]["out"], dtype=np.float32)
    if _trace:
        kernel.last_results = res
    return out


kernel.last_results = None


# revision 10
# speedup vs baseline: 1.1081x; 1.0134x over previous
"""Trainium2 Bass kernel v3 for nn_CRAP_16544214024675 (sparse_attention).

Reference computation (per batch b, channel c):
  q = Wq@feat + bq                        (1x1 conv over channels)
  k = unfold3x3_s2(src)                   (strided window gather, pad 1)
  v = unfold3x3_s2(Wv@src + bv)
  A = softmax_t( sum_px q*k_t / 64 )      (9 window positions)
  out = fold3x3_s1( A_t * v_t ) * feat

Sharding: 8 cores = 4 batches x 2 output-channel halves (slot 0 of packed
inputs = the core's own half; channel sums commute).

v3 vs v2 (79.6us baseline): full reschedule for overlap.
 - DMA: featb as 4 row-band transfers (early q-conv start), then src
   parity planes ct0-first (products are the long pole), ct1 planes after,
   ct1 of plane (0,0) last (its vplane feeds only the late fold passes).
 - products: 7 DVE STT + 2 gpsimd mul (plane-(1,1) t's) with ACT reduces.
   No shifted-q copy (STT runs 1x, no alignment constraint).
 - fold in two half-image PSUM phases (4 banks each): half0 coexists with
   the conv chunk pool (2x2 banks) and accumulates t-by-t as exps arrive;
   half1 reuses the conv banks after convs drain. Zero-prefill per half
   keeps accumulation order free.
 - v-convs K-split: ct0 matmuls run as soon as the ct0 plane lands, ct1
   matmuls chase the late DMA.
 - final out = (fold*rz)*feat per 16-row band: bands 0-1 via ACT scale +
   DVE mul, bands 2-3 via one DVE STT each; band DMA-out chases.
"""
import sys
from contextlib import ExitStack

import numpy as np

for _p in ("/opt/trn_rl_repo", "/root/.axon_site/_ro/trn_rl_repo"):
    if _p not in sys.path:
        sys.path.append(_p)

import ml_dtypes

import concourse.tile as tile
from concourse import bacc, mybir
from concourse import bass_utils
from concourse.bass_interp import get_hw_module

F32 = mybir.dt.float32
BF16 = mybir.dt.bfloat16
AF = mybir.ActivationFunctionType
ALU = mybir.AluOpType

B, C, H, W = 4, 256, 64, 64
N_CORES = 8

# src parity plane (p,q) -> t's whose product reads its ct0 and whose fold
# reads vplane (p,q)
PLANE_TS = {
    (0, 0): [(1, 1)],
    (1, 1): [(2, 2), (0, 0), (0, 2), (2, 0)],
    (1, 0): [(0, 1), (2, 1)],
    (0, 1): [(1, 0), (1, 2)],
}
# DMA arrival order of (plane, ct): all ct0 first (product chain), then
# ct1; plane (0,0) ct1 last (only feeds the final fold passes), split in 2.
DMA_PLANES = [
    ((0, 0), 0), ((0, 0), 1), ((1, 1), 0), ((1, 1), 1),
    ((1, 0), 0), ((0, 1), 0), ((1, 0), 1), ((0, 1), 1),
]
CONV_ORDER = [(0, 0), (1, 1), (1, 0), (0, 1)]
# per-t product engine: 'S' DVE STT fused; 'A' DVE 2x mul + ACT reduce
# (A-type t's have j in {1,2} so both product views stay alignable)
T_TYPE = {
    (1, 1): 'S', (2, 2): 'A', (0, 0): 'S', (0, 2): 'S', (2, 0): 'S',
    (0, 1): 'A', (2, 1): 'A', (1, 0): 'S', (1, 2): 'S',
}
# product emission order (DVE/GPS priority); follows ct0 arrival
T_ORDER = [(1, 1), (2, 2), (0, 0), (2, 0), (0, 2),
           (0, 1), (2, 1), (1, 0), (1, 2)]
T_IDX = {t: i for i, t in enumerate(T_ORDER)}
# fold emission order per half = expected readiness order (WAW serialized)
FOLD_ORDER = [(1, 1), (2, 2), (0, 0), (0, 2), (2, 0),
              (0, 1), (1, 0), (2, 1), (1, 2)]
N_WARM = 55
N_FILL = 12


def _prod_views(i, j):
    qr0, pr0, rows = (1, 0, 63) if i == 0 else (0, 0, 64)
    qc0, pc0, cols = (1, 0, 63) if j == 0 else (0, 0, 64)
    return qr0, pr0, rows, qc0, pc0, cols


def _windows(i, j):
    if i == 0:
        yo0, yo1, dy = 0, 63, 0
    elif i == 1:
        yo0, yo1, dy = 0, 64, 0
    else:
        yo0, yo1, dy = 1, 64, -1
    if j == 0:
        xo0, xo1, dx = 0, 63, 0
    elif j == 1:
        xo0, xo1, dx = 0, 64, 0
    else:
        xo0, xo1, dx = 1, 64, -1
    return yo0, yo1, dy, xo0, xo1, dx


def build_program():
    nc = bacc.Bacc("TRN2", target_bir_lowering=False, debug=False)

    fb_d = nc.dram_tensor("featbands", (4, 2, 128, 16, W), BF16,
                          kind="ExternalInput")
    spl_d = nc.dram_tensor("splanes", (8, 128, H, W), BF16,
                           kind="ExternalInput")
    wq_d = nc.dram_tensor("wq", (2, 128, 128), BF16, kind="ExternalInput")
    wv_d = nc.dram_tensor("wv", (2, 128, 128), BF16, kind="ExternalInput")
    bq_d = nc.dram_tensor("bq", (128, 1), F32, kind="ExternalInput")
    bv_d = nc.dram_tensor("bv", (128, 1), F32, kind="ExternalInput")
    id_d = nc.dram_tensor("identb", (128, 128), BF16, kind="ExternalInput")
    out_d = nc.dram_tensor("out", (128, H, W), BF16, kind="ExternalOutput")

    with tile.TileContext(nc) as tc, ExitStack() as ctx:
        pool = ctx.enter_context(tc.tile_pool(name="main", bufs=1))
        scpool = ctx.enter_context(tc.tile_pool(name="scratch", bufs=3))
        wtpool = ctx.enter_context(tc.tile_pool(name="wts", bufs=9))

        # --- input DMA: emission order = priority order ---
        wq_t = pool.tile([128, 2, 128], BF16, tag="wq")
        wv_t = pool.tile([128, 2, 128], BF16, tag="wv")
        bq_t = pool.tile([128, 1], F32, tag="bq")
        bv_t = pool.tile([128, 1], F32, tag="bv")
        nc.sync.dma_start(wq_t[:], wq_d.ap().rearrange("a p b -> p a b"))
        nc.sync.dma_start(bq_t[:], bq_d.ap())
        fbt = []
        for b in range(4):
            t_ = pool.tile([128, 2, 16, W], BF16, tag=f"fb{b}")
            nc.sync.dma_start(t_[:], fb_d.ap()[b].rearrange("h p r c -> p h r c"))
            fbt.append(t_)
        nc.sync.dma_start(wv_t[:], wv_d.ap().rearrange("a p b -> p a b"))
        nc.sync.dma_start(bv_t[:], bv_d.ap())
        id_t = pool.tile([128, 128], BF16, tag="identb")
        nc.sync.dma_start(id_t[:], id_d.ap())
        splane = {}
        spl00h = []
        for si, (pq, ct) in enumerate(DMA_PLANES):
            if si == 0:
                for hh in range(2):
                    th = pool.tile([128, 32, W], BF16, tag=f"spl00h{hh}",
                                   name=f"spl00h{hh}")
                    nc.sync.dma_start(
                        th[:], spl_d.ap()[si, :, 32 * hh:32 * hh + 32, :])
                    spl00h.append(th)
                continue
            t_ = pool.tile([128, H, W], BF16, tag=f"spl{ct}{pq[0]}{pq[1]}",
                           name=f"spl{ct}{pq[0]}{pq[1]}")
            splane[(ct,) + pq] = t_
            nc.sync.dma_start(t_[:], spl_d.ap()[si])

        lg_t = pool.tile([128, 16], F32, tag="lg")
        exp_t = pool.tile([128, 16], F32, tag="exp")
        q_t = pool.tile([128, H, W], BF16, tag="q")
        zero_t = pool.tile([128, 128], BF16, tag="zero")
        nc.gpsimd.memset(zero_t[:], 0.0)
        warm_t = pool.tile([128, 256], BF16, tag="warm")
        nc.gpsimd.memset(warm_t[:], 0.5)

        # trigger the ACT exp-table load immediately (off the critical path)
        nc.gpsimd.memset(lg_t[:, 15:16], 0.0)
        with tc.high_priority():
            nc.scalar.activation(exp_t[:, 15:16], lg_t[:, 15:16], AF.Exp,
                                 scale=1.0 / 64.0)

        # PSUM: conv chunks 2x[128,16,W] (4 banks) + fold half0 (4 banks);
        # fold half1 opens after the conv pool closes.
        psc_cm = tc.tile_pool(name="psc", bufs=2, space="PSUM")
        psfA = ctx.enter_context(
            tc.tile_pool(name="psfA", bufs=1, space="PSUM"))
        fold_h = [psfA.tile([128, 32, W], F32, tag="foldh0", name="foldh0")]

        def fold_pass(t_ij, half, stop):
            i, j = t_ij
            yo0, yo1, dy, xo0, xo1, dx = _windows(i, j)
            pq = ((i + 1) % 2, (j + 1) % 2)
            vp = vplane[pq]
            dg = diags[(i, j)]
            fps = fold_h[half]
            r0 = 32 * half
            yb = max(yo0, r0)
            yend = min(yo1, r0 + 32)
            while yb < yend:
                ye = min((yb // 8 + 1) * 8, yend)
                nc.tensor.matmul(
                    fps[:, yb - r0:ye - r0, xo0:xo1],
                    dg[:],
                    vp[:, yb + dy:ye + dy, xo0 + dx:xo1 + dx],
                    start=(t_ij == (1, 1)), stop=stop,
                    skip_group_check=True,
                )
                yb = ye

        vplane = {}
        diags = {}
        with psc_cm as psc:
            # PE warm-up burst (dependency-free) to ramp the clock
            wps = psc.tile([128, 16, W], F32, tag="cv", name="warmps")
            for w_i in range(N_WARM):
                nc.tensor.matmul(
                    wps[:, 0:2, :], warm_t[:, 0:128], warm_t[:, 128:256],
                    start=True, stop=True, skip_group_check=True,
                )

            # q-conv: 4 row-band chunks chasing the featb DMAs
            for b in range(4):
                ps = psc.tile([128, 16, W], F32, tag="cv", name=f"qps{b}")
                for s in range(2):
                    for k in range(2):
                        nc.tensor.matmul(
                            ps[:, 8 * s:8 * s + 8, :],
                            wq_t[:, k, :],
                            fbt[b][:, k, 8 * s:8 * s + 8, :],
                            start=(k == 0), stop=(k == 1),
                        )
                with tc.high_priority(offset=60):
                    nc.scalar.activation(q_t[:, 16 * b:16 * b + 16, :],
                                         ps[:], AF.Identity, bias=bq_t[:])


            # --- products ---
            def product(t_ij):
                i, j = t_ij
                idx = T_IDX[t_ij]
                pq = [k for k, ts in PLANE_TS.items() if t_ij in ts][0]
                sc = scpool.tile([128, H, W], BF16, tag="prod",
                                 name=f"prod{idx}")
                if t_ij == (1, 1):
                    # row-half STTs chasing the split plane-(0,0) DMA
                    for hh in range(2):
                        with tc.high_priority(offset=50):
                            nc.vector.scalar_tensor_tensor(
                                out=sc[:, 32 * hh:32 * hh + 32, :],
                                in0=q_t[:, 32 * hh:32 * hh + 32, :],
                                scalar=1.0, in1=spl00h[hh][:],
                                op0=ALU.mult, op1=ALU.mult,
                                accum_out=lg_t[:, idx + 9 * hh:idx + 9 * hh + 1])
                    with tc.high_priority(offset=50):
                        nc.vector.tensor_tensor(
                            out=lg_t[:, idx:idx + 1],
                            in0=lg_t[:, idx:idx + 1],
                            in1=lg_t[:, idx + 9:idx + 10], op=ALU.add)
                    with tc.high_priority():
                        nc.scalar.activation(
                            exp_t[:, idx:idx + 1], lg_t[:, idx:idx + 1],
                            AF.Exp, scale=1.0 / 64.0)
                    dg = wtpool.tile([128, 128], BF16, tag="wt",
                                     name=f"dg{idx}")
                    diags[t_ij] = dg
                    with tc.high_priority():
                        nc.vector.tensor_scalar(
                            out=dg[:], in0=id_t[:],
                            scalar1=exp_t[:, idx:idx + 1], scalar2=None,
                            op0=ALU.mult)
                    return
                pl = splane[(0,) + pq]
                qr0, pr0, rows, qc0, pc0, cols = _prod_views(i, j)
                q_view = q_t[:, qr0:qr0 + rows, qc0:qc0 + cols]
                p_view = pl[:, pr0:pr0 + rows, pc0:pc0 + cols]
                if T_TYPE[t_ij] == 'A':
                    with tc.high_priority(offset=50):
                        nc.vector.tensor_mul(sc[:, 0:rows, 0:cols],
                                             q_view, p_view)
                    with tc.high_priority(offset=50):
                        nc.scalar.activation(
                            sc[:, 0:rows, 0:cols], sc[:, 0:rows, 0:cols],
                            AF.Copy, accum_out=lg_t[:, idx:idx + 1])
                else:
                    with tc.high_priority(offset=50):
                        nc.vector.scalar_tensor_tensor(
                            out=sc[:, 0:rows, 0:cols], in0=q_view,
                            scalar=1.0, in1=p_view,
                            op0=ALU.mult, op1=ALU.mult,
                            accum_out=lg_t[:, idx:idx + 1])
                with tc.high_priority():
                    nc.scalar.activation(
                        exp_t[:, idx:idx + 1], lg_t[:, idx:idx + 1],
                        AF.Exp, scale=1.0 / 64.0)
                dg = wtpool.tile([128, 128], BF16, tag="wt",
                                 name=f"dg{idx}")
                diags[t_ij] = dg
                with tc.high_priority():
                    nc.vector.tensor_scalar(
                        out=dg[:], in0=id_t[:],
                        scalar1=exp_t[:, idx:idx + 1], scalar2=None,
                        op0=ALU.mult)

            for t_ij in T_ORDER:
                product(t_ij)

            # --- v-convs (K-split: ct0 part early, ct1 part chases DMA) ---
            def conv_plane(pq):
                vp = pool.tile([128, H, W], BF16, tag=f"vpl{pq[0]}{pq[1]}",
                               name=f"vpl{pq[0]}{pq[1]}")
                vplane[pq] = vp
                for half in range(2):
                    ps = psc.tile([128, 16, W], F32, tag="cv",
                                  name=f"cv{pq[0]}{pq[1]}h{half}a")
                    ps2 = psc.tile([128, 16, W], F32, tag="cv",
                                   name=f"cv{pq[0]}{pq[1]}h{half}b")
                    for ci, ps_ in enumerate((ps, ps2)):
                        r0 = 32 * half + 16 * ci
                        for s in range(2):
                            for k in range(2):
                                if k == 0 and pq == (0, 0):
                                    rr = r0 + 8 * s
                                    src_ap = spl00h[rr // 32][
                                        :, rr % 32:rr % 32 + 8, :]
                                else:
                                    src_ap = splane[(k,) + pq][
                                        :, r0 + 8 * s:r0 + 8 * s + 8, :]
                                nc.tensor.matmul(
                                    ps_[:, 8 * s:8 * s + 8, :],
                                    wv_t[:, k, :],
                                    src_ap,
                                    start=(k == 0), stop=(k == 1),
                                )
                    with tc.high_priority(offset=30):
                        nc.scalar.activation(
                            vp[:, 32 * half:32 * half + 16, :], ps[:],
                            AF.Identity, bias=bv_t[:])
                        nc.scalar.activation(
                            vp[:, 32 * half + 16:32 * half + 32, :], ps2[:],
                            AF.Identity, bias=bv_t[:])

            for ci, pq in enumerate(CONV_ORDER):
                conv_plane(pq)
                if ci == 0:
                    fps_ = psc.tile([128, 16, W], F32, tag="cv",
                                    name="fillps")
                    for _f in range(N_FILL):
                        nc.tensor.matmul(
                            fps_[:, 0:2, :], warm_t[:, 0:128],
                            warm_t[:, 128:256],
                            start=True, stop=True, skip_group_check=True,
                        )


        # conv pool closed -> fold half1 banks free
        psfB = ctx.enter_context(
            tc.tile_pool(name="psfB", bufs=1, space="PSUM"))
        fold_h.append(psfB.tile([128, 32, W], F32, tag="foldh1", name="foldh1"))
        for t_ij in FOLD_ORDER:
            fold_pass(t_ij, 0, stop=(t_ij == FOLD_ORDER[-1]))
            fold_pass(t_ij, 1, stop=(t_ij == FOLD_ORDER[-1]))

        # --- normalization + final: out = (fold * rz) * feat, per band ---
        z_t = pool.tile([128, 8], F32, tag="z")
        rz_t = pool.tile([128, 1], F32, tag="rz")
        with tc.high_priority():
            nc.vector.tensor_reduce(z_t[:, 0:1], exp_t[:, 0:9],
                                    axis=mybir.AxisListType.X, op=ALU.add)
            nc.vector.reciprocal(rz_t[:], z_t[:, 0:1])
        out_t = pool.tile([128, H, W], BF16, tag="out")
        scf = [pool.tile([128, 16, W], BF16, tag=f"scf{i}", name=f"scf{i}") for i in range(2)]
        for band in range(4):
            half, sub = divmod(band, 2)
            fview = fold_h[half][:, 16 * sub:16 * sub + 16, :]
            feat_b = fbt[band][:, 0, :, :]
            oview = out_t[:, 16 * band:16 * band + 16, :]
            if half == 0:
                with tc.high_priority(offset=20):
                    nc.scalar.activation(scf[sub][:], fview,
                                         AF.Identity, scale=rz_t[:])
                    nc.vector.tensor_mul(oview, scf[sub][:], feat_b)
            else:
                with tc.high_priority(offset=20):
                    nc.vector.scalar_tensor_tensor(
                        out=oview, in0=fview, scalar=rz_t[:, 0:1],
                        in1=feat_b, op0=ALU.mult, op1=ALU.mult)
            nc.sync.dma_start(out_d.ap()[:, 16 * band:16 * band + 16, :],
                              oview)

    nc.compile()
    nc.m = get_hw_module(nc.m)
    return nc


_PROGRAM = None


def _get_program():
    global _PROGRAM
    if _PROGRAM is None:
        _PROGRAM = build_program()
    return _PROGRAM


def _prep_inputs(feat, src, Wq, bq, Wv, bv):
    bf = ml_dtypes.bfloat16
    # src parity planes: (B, ct, p, q, 128, H, W); ct0 = own half
    spl = np.ascontiguousarray(
        src.reshape(B, 2, 128, H, 2, W, 2).transpose(0, 1, 4, 6, 2, 3, 5)
    ).astype(bf)
    featb = feat.reshape(B, 2, 128, H, W).astype(bf)
    wq3 = np.ascontiguousarray(Wq.T).reshape(2, 128, C)
    wv3 = np.ascontiguousarray(Wv.T).reshape(2, 128, C)
    identb = np.eye(128, dtype=np.float32).astype(bf)
    in_maps = []
    for core in range(N_CORES):
        b, h = divmod(core, 2)
        oc = slice(h * 128, h * 128 + 128)
        order = [h, 1 - h]  # slot 0 = own input-channel half
        # featbands: (4, 2, 128, 16, W) - row-band major, slot0 = own half
        fb = np.ascontiguousarray(
            featb[b][order].reshape(2, 128, 4, 16, W).transpose(2, 0, 1, 3, 4))
        spl_core = np.stack([
            spl[b][order[ct]][p][q] for (p, q), ct in DMA_PLANES
        ])
        in_maps.append(
            dict(
                featbands=fb,
                splanes=np.ascontiguousarray(spl_core),
                wq=np.ascontiguousarray(wq3[order][:, :, oc]).astype(bf),
                wv=np.ascontiguousarray(wv3[order][:, :, oc]).astype(bf),
                bq=bq[oc].reshape(128, 1).astype(np.float32),
                bv=bv[oc].reshape(128, 1).astype(np.float32),
                identb=identb,
            )
        )
    return in_maps


def kernel(feat, src, Wq, bq, Wv, bv, _trace=False):
    feat = np.asarray(feat, np.float32)
    src = np.asarray(src, np.float32)
    Wq = np.asarray(Wq, np.float32)
    bq = np.asarray(bq, np.float32)
    Wv = np.asarray(Wv, np.float32)
    bv = np.asarray(bv, np.float32)

    in_maps = _prep_inputs(feat, src, Wq, bq, Wv, bv)
    nc = _get_program()
    res = bass_utils.run_bass_kernel_spmd(
        nc, in_maps, core_ids=list(range(N_CORES)), trace=_trace
    )
    out = np.empty((B, C, H, W), np.float32)
    for core in range(N_CORES):
        b, h = divmod(core, 2)
        out[b, h * 128:h * 128 + 128] = np.asarray(
            res.results[core]["out"], dtype=np.float32)
    if _trace:
        kernel.last_results = res
    return out


kernel.last_results = None
